# revision 15
# baseline (speedup 1.0000x reference)
"""Trainium2 Bass kernel for i1e(z) (exponentially scaled modified Bessel I1).

Input: z float32 (32, 1024, 1024), values in [0.1, 10.1] (positive).
Output: i1e(z), same shape/dtype, matching the A&S-style reference to
~9e-3 max pointwise / ~5.7e-3 normed relative error.

Strategy (per core, trivially data-parallel over the leading batch axis):
  Each of 8 cores gets 4 batches = 4Mi elements, viewed as [128, 32768] f32.

  i1e(x) = exp(g(ln x)) where g(v) = ln(i1e(e^v)) is asymptotically LINEAR
  in v at both ends (g ~ v + const as x->0, g ~ -v/2 + const as x->inf), so
  a degree-4 minimax polynomial hits 8.0e-3 max error over x in [0.1, 10.1].

  Per tile, one table set (natural_log_exp_and_others) on ScalarE and only
  fp16 fast-mode ops on VectorE:
    ACT:  v   = Ln(x)                 -> fp16        (1x, ~29us/core)
    ACT:  S   = Square(alpha*v+beta)  -> fp16        (head: a4 v^2 + a3 v)
    TS :  acc = S + delta                            (fp16 4x, ~9us/core)
    TT :  acc = acc * v                              (fp16 2x, ~18us/core)
    TS :  acc = acc + a1
    TT :  acc = acc * v
    ACT:  out = Exp(acc + a0)         -> f32
  DVE total ~50us/core, ACT ~65us/core, DMA (16MiB in + 16MiB out) ~60us
  per core -> wall ~65us, against a measured pure-DMA floor of ~59us.

  No branches: the old two-branch A&S evaluation needed 8 ACT passes and
  10 DVE passes that all run at 1x (scalar_tensor_tensor/copy_predicated
  have no fast DVE perf modes; measured 494us/run); this formulation needs
  3 ACT + 4 DVE passes, all in fast modes (measured ~65us/run).
  A degree-5 variant (one more TT+TS pair, 4.0e-3 norm error) measured
  ~73us; degree-4 with only Ln+Exp on ACT (3 TT) also measured ~73us.
"""

import numpy as np

import concourse.bass as bass
import concourse.tile as tile
from concourse import mybir
from concourse.bass_utils import run_bass_kernel_spmd

AF = mybir.ActivationFunctionType
ALU = mybir.AluOpType
F32 = mybir.dt.float32
F16 = mybir.dt.float16

N_CORES = 8
P = 128              # SBUF partitions
FD_TOTAL = 32768     # free-dim elements per partition per core (4Mi total)
TILE_FD = 4096       # free-dim per tile
N_TILES = FD_TOTAL // TILE_FD

# Degree-4 minimax fit of g(v) = ln(i1e(e^v)) on v in [ln 0.1, ln 10.1],
# max |p - g| = 7.98e-3.  p(v) = sum a[k] v^k.  The top three coefficients
# are folded into one ACT Square (completed square):
# Square(alpha*v + beta) + delta = a4 v^2 + a3 v + a2
A0 = -1.5758923301576444
A1 = 0.22380646428888462
ALPHA = 0.1034912154645873       # sqrt(a4)
BETA = -0.012173864918938996     # a3 / (2 alpha)
DELTA = -0.2504574187620049      # a2 - beta^2

ACT_BIAS_CONSTS = [BETA, A0]

_CACHED_NC = None


def build_nc(reps: int = 1):
    nc = bass.Bass(trn_type="TRN2")
    x_ext = nc.declare_dram_parameter("x", [P, FD_TOTAL], F32, isOutput=False)
    o_ext = nc.declare_dram_parameter("o", [P, FD_TOTAL], F32, isOutput=True)

    # Register activation-bias constants as const APs, mirroring
    # Bass.__init__'s register_const_ap for 0.0/1.0.
    for i, val in enumerate(ACT_BIAS_CONSTS):
        tns = nc.alloc_sbuf_tensor(f"const-f32-bias{i}", [P, 1], F32)
        nc.gpsimd.memset(tns.ap(), val)
        nc.const_aps.aps[(F32, val)] = tns.ap()
    nc.all_engine_barrier()

    # Dummy 1-element activation: triggers the natural_log_exp_and_others
    # ACT_TABLE_LOAD (~2.7us) now, overlapping it with the first input DMA
    # instead of serializing after it.
    warm = nc.alloc_sbuf_tensor("act-table-warm", [P, 1], F32)
    nc.scalar.activation(warm.ap(), nc.const_aps.aps[(F32, ACT_BIAS_CONSTS[0])],
                         AF.Exp)

    with tile.TileContext(nc) as tc:
        with (
            tc.tile_pool(name="io", bufs=3) as io,
            tc.tile_pool(name="tmp", bufs=2) as tmp,
        ):
            for i in range(N_TILES * reps):
                i = i % N_TILES
                sl = bass.ts(i, TILE_FD)

                x = io.tile([P, TILE_FD], F32, tag="x")
                nc.sync.dma_start(x[:], x_ext[:, sl])

                # ScalarE (one table set): v = ln x, S = (alpha*v+beta)^2
                v = tmp.tile([P, TILE_FD], F16, tag="v")
                nc.scalar.activation(v[:], x[:], AF.Ln)
                s = tmp.tile([P, TILE_FD], F16, tag="s")
                nc.scalar.activation(s[:], v[:], AF.Square,
                                     scale=ALPHA, bias=BETA)

                # VectorE: fp16 Horner, adds in 4x tensor_scalar,
                # mults in 2x tensor_tensor.
                acc = tmp.tile([P, TILE_FD], F16, tag="acc")
                nc.vector.tensor_scalar_add(acc[:], s[:], DELTA)
                nc.vector.tensor_tensor(acc[:], acc[:], v[:], ALU.mult)
                nc.vector.tensor_scalar_add(acc[:], acc[:], A1)
                nc.vector.tensor_tensor(acc[:], acc[:], v[:], ALU.mult)

                # ScalarE: out = exp(acc + a0) -> f32
                out = io.tile([P, TILE_FD], F32, tag="out")
                nc.scalar.activation(out[:], acc[:], AF.Exp, bias=A0)

                nc.sync.dma_start(o_ext[:, sl], out[:])

    _split_multi_waits(nc)
    return nc


# TPB compute-instruction ISA formats carry at most ONE sync-wait, but Tile's
# semaphore assignment can attach several (its wait minimality is per-proc,
# not transitive).  Hoist all but one wait onto an InstNoOp inserted right
# before the offending instruction on the same engine.
def _split_multi_waits(nc):
    for bb in nc.main_func.blocks:
        insts = bb.instructions
        i = 0
        while i < len(insts):
            inst = insts[i]
            si = inst.sync_info
            if si is not None and len(si.on_wait) > 1:
                for w in si.on_wait[:-1]:
                    nop = mybir.InstNoOp(
                        name=nc.get_next_instruction_name(),
                        text_hint="wait_split",
                        bass_nofuse=True,
                        engine=inst.engine,
                        sync_info=mybir.SyncInfo(on_wait=[w], on_update=[]),
                    )
                    insts.insert(i, nop)
                    i += 1
                si.on_wait = [si.on_wait[-1]]
            i += 1


def make_in_maps(z: np.ndarray) -> list:
    per_core = 32 // N_CORES
    shards = z.reshape(N_CORES, per_core * 1024 * 1024).reshape(N_CORES, P, FD_TOTAL)
    return [{"x": np.ascontiguousarray(shards[k])} for k in range(N_CORES)]


def kernel(z: np.ndarray) -> np.ndarray:
    global _CACHED_NC
    assert z.shape == (32, 1024, 1024) and z.dtype == np.float32
    if _CACHED_NC is None:
        _CACHED_NC = build_nc()
    nc = _CACHED_NC

    per_core = 32 // N_CORES
    in_maps = make_in_maps(z)
    res = run_bass_kernel_spmd(nc, in_maps, list(range(N_CORES))).results
    out = np.concatenate(
        [res[k]["o"].reshape(per_core, 1024, 1024) for k in range(N_CORES)], axis=0
    )
    return out.astype(np.float32)


# revision 17
# speedup vs baseline: 1.3332x; 1.3332x over previous
"""Trainium2 Bass kernel for i1e(z) (exponentially scaled modified Bessel I1).

Input: z float32 (32, 1024, 1024), values in [0.1, 10.1] (positive).
Output: i1e(z), same shape/dtype, matching the A&S-style reference to
~9e-3 max pointwise / ~5.7e-3 normed relative error.

Strategy (per core, trivially data-parallel over the leading batch axis):
  Each of 8 cores gets 4 batches = 4Mi elements, viewed as [128, 32768] f32.

  i1e(x) = exp(g(ln x)) where g(v) = ln(i1e(e^v)) is asymptotically LINEAR
  in v at both ends (g ~ v + const as x->0, g ~ -v/2 + const as x->inf), so
  a degree-4 minimax polynomial hits 8.0e-3 max error over x in [0.1, 10.1].

  Per tile, one table set (natural_log_exp_and_others) on ScalarE and only
  fp16 fast-mode ops on VectorE:
    ACT:  v   = Ln(x)                 -> fp16        (1x, ~29us/core)
    ACT:  S   = Square(alpha*v+beta)  -> fp16        (head: a4 v^2 + a3 v)
    TS :  acc = S + delta                            (fp16 4x, ~9us/core)
    TT :  acc = acc * v                              (fp16 2x, ~18us/core)
    TS :  acc = acc + a1
    TT :  acc = acc * v
    ACT:  out = Exp(acc + a0)         -> f32
  DVE total ~50us/core, ACT ~65us/core, DMA (16MiB in + 16MiB out) ~60us
  per core -> wall ~65us, against a measured pure-DMA floor of ~59us.

  No branches: the old two-branch A&S evaluation needed 8 ACT passes and
  10 DVE passes that all run at 1x (scalar_tensor_tensor/copy_predicated
  have no fast DVE perf modes; measured 494us/run); this formulation needs
  3 ACT + 4 DVE passes, all in fast modes (measured ~65us/run).
  A degree-5 variant (one more TT+TS pair, 4.0e-3 norm error) measured
  ~73us; degree-4 with only Ln+Exp on ACT (3 TT) also measured ~73us.
"""

import numpy as np

import concourse.bass as bass
import concourse.tile as tile
from concourse import mybir
from concourse.bass_utils import run_bass_kernel_spmd

AF = mybir.ActivationFunctionType
ALU = mybir.AluOpType
F32 = mybir.dt.float32
F16 = mybir.dt.float16

N_CORES = 8
P = 128              # SBUF partitions
FD_TOTAL = 32768     # free-dim elements per partition per core (4Mi total)
TILE_FD = 4096       # free-dim per tile
N_TILES = FD_TOTAL // TILE_FD

# Degree-4 minimax fit of g(v) = ln(i1e(e^v)) on v in [ln 0.1, ln 10.1],
# max |p - g| = 7.98e-3.  p(v) = sum a[k] v^k.  The top three coefficients
# are folded into one ACT Square (completed square):
# Square(alpha*v + beta) + delta = a4 v^2 + a3 v + a2
A0 = -1.5758923301576444
A1 = 0.22380646428888462
ALPHA = 0.1034912154645873       # sqrt(a4)
BETA = -0.012173864918938996     # a3 / (2 alpha)
DELTA = -0.2504574187620049      # a2 - beta^2
# Same head quadratic in factored form a4*(v-R1)*(v-R2), used on the tiles
# whose head runs on the (otherwise slack) VectorE instead of ScalarE:
A4 = 0.010710431678337632
R1 = 4.953377659533114
R2 = -4.7181139250356745
# Per 8 tiles, 3 use the DVE head (3 TT + 3 TS) and 5 the ACT Square head
# (2 TT + 2 TS + 1 ACT) -- balances ScalarE (21 tile-passes) against
# VectorE so neither engine sits far above the DMA floor.
DVE_HEAD = frozenset((0, 3, 6))

ACT_BIAS_CONSTS = [BETA, A0]

_CACHED_NC = None


def build_nc(reps: int = 1):
    nc = bass.Bass(trn_type="TRN2")
    x_ext = nc.declare_dram_parameter("x", [P, FD_TOTAL], F32, isOutput=False)
    o_ext = nc.declare_dram_parameter("o", [P, FD_TOTAL], F32, isOutput=True)

    # Register activation-bias constants as const APs, mirroring
    # Bass.__init__'s register_const_ap for 0.0/1.0.
    for i, val in enumerate(ACT_BIAS_CONSTS):
        tns = nc.alloc_sbuf_tensor(f"const-f32-bias{i}", [P, 1], F32)
        nc.gpsimd.memset(tns.ap(), val)
        nc.const_aps.aps[(F32, val)] = tns.ap()
    nc.all_engine_barrier()

    # Dummy 1-element activation: triggers the natural_log_exp_and_others
    # ACT_TABLE_LOAD (~2.7us) now, overlapping it with the first input DMA
    # instead of serializing after it.
    warm = nc.alloc_sbuf_tensor("act-table-warm", [P, 1], F32)
    nc.scalar.activation(warm.ap(), nc.const_aps.aps[(F32, ACT_BIAS_CONSTS[0])],
                         AF.Exp)

    with tile.TileContext(nc) as tc:
        with (
            tc.tile_pool(name="io", bufs=3) as io,
            tc.tile_pool(name="tmp", bufs=2) as tmp,
        ):
            for i in range(N_TILES * reps):
                i = i % N_TILES
                sl = bass.ts(i, TILE_FD)

                x = io.tile([P, TILE_FD], F32, tag="x")
                nc.sync.dma_start(x[:], x_ext[:, sl])

                # ScalarE (one table set): v = ln x
                v = tmp.tile([P, TILE_FD], F16, tag="v")
                nc.scalar.activation(v[:], x[:], AF.Ln)

                # Head quadratic a4 v^2 + a3 v + a2, on ScalarE (Square of
                # the completed square) or VectorE (factored real roots),
                # chosen per tile to balance the two engines.
                acc = tmp.tile([P, TILE_FD], F16, tag="acc")
                if i in DVE_HEAD:
                    t2 = tmp.tile([P, TILE_FD], F16, tag="t2")
                    nc.vector.tensor_scalar(acc[:], v[:], -R1, A4,
                                            ALU.add, ALU.mult)
                    nc.vector.tensor_scalar_add(t2[:], v[:], -R2)
                    nc.vector.tensor_tensor(acc[:], acc[:], t2[:], ALU.mult)
                else:
                    s = tmp.tile([P, TILE_FD], F16, tag="s")
                    nc.scalar.activation(s[:], v[:], AF.Square,
                                         scale=ALPHA, bias=BETA)
                    nc.vector.tensor_scalar_add(acc[:], s[:], DELTA)

                # VectorE: fp16 Horner tail, adds in 4x tensor_scalar,
                # mults in 2x tensor_tensor.
                nc.vector.tensor_tensor(acc[:], acc[:], v[:], ALU.mult)
                nc.vector.tensor_scalar_add(acc[:], acc[:], A1)
                nc.vector.tensor_tensor(acc[:], acc[:], v[:], ALU.mult)

                # ScalarE: out = exp(acc + a0) -> f32
                out = io.tile([P, TILE_FD], F32, tag="out")
                nc.scalar.activation(out[:], acc[:], AF.Exp, bias=A0)

                nc.sync.dma_start(o_ext[:, sl], out[:])

    _split_multi_waits(nc)
    return nc


# TPB compute-instruction ISA formats carry at most ONE sync-wait, but Tile's
# semaphore assignment can attach several (its wait minimality is per-proc,
# not transitive).  Hoist all but one wait onto an InstNoOp inserted right
# before the offending instruction on the same engine.
def _split_multi_waits(nc):
    for bb in nc.main_func.blocks:
        insts = bb.instructions
        i = 0
        while i < len(insts):
            inst = insts[i]
            si = inst.sync_info
            if si is not None and len(si.on_wait) > 1:
                for w in si.on_wait[:-1]:
                    nop = mybir.InstNoOp(
                        name=nc.get_next_instruction_name(),
                        text_hint="wait_split",
                        bass_nofuse=True,
                        engine=inst.engine,
                        sync_info=mybir.SyncInfo(on_wait=[w], on_update=[]),
                    )
                    insts.insert(i, nop)
                    i += 1
                si.on_wait = [si.on_wait[-1]]
            i += 1


def make_in_maps(z: np.ndarray) -> list:
    per_core = 32 // N_CORES
    shards = z.reshape(N_CORES, per_core * 1024 * 1024).reshape(N_CORES, P, FD_TOTAL)
    return [{"x": np.ascontiguousarray(shards[k])} for k in range(N_CORES)]


def kernel(z: np.ndarray) -> np.ndarray:
    global _CACHED_NC
    assert z.shape == (32, 1024, 1024) and z.dtype == np.float32
    if _CACHED_NC is None:
        _CACHED_NC = build_nc()
    nc = _CACHED_NC

    per_core = 32 // N_CORES
    in_maps = make_in_maps(z)
    res = run_bass_kernel_spmd(nc, in_maps, list(range(N_CORES))).results
    out = np.concatenate(
        [res[k]["o"].reshape(per_core, 1024, 1024) for k in range(N_CORES)], axis=0
    )
    return out.astype(np.float32)
